# revision 6
# baseline (speedup 1.0000x reference)
"""Additive attention kernel for Trainium2 (8 NeuronCores, SPMD).

Reference computation (B=4, L=1024, D=256, U=128):
    q = X @ W1 + b1                              [B,L,U]
    k = X @ W2                                   [B,L,U]
    g = tanh(q[:,:,None,:] + k[:,None,:,:])      [B,L,L,U]
    s = sigmoid(g @ W3 + b2)                     [B,L,L]
    out = s @ X                                  [B,L,D]

Key idea: replace the O(L*L*U) tanh evaluation with a rank-R functional
decomposition (barycentric Lagrange interpolation in the tanh shift):

    tanh(q + k) ~= sum_r  ell_r(q) * tanh(k + s_r)

with s_r Chebyshev nodes on [-C, C] and ell_r(q) = coef_r * prod_{j!=r}
(clip(q) - s_j).  The score then becomes R=16 rank-U matmuls per key
block on the PE instead of 67M tanh evaluations on ACT:

    psT[kk,q] = sum_r sum_u (W3_u ell_r(q_u)) * tanh(k_u(kk) + s_r)

Leave-one-out products come from division-free prefix/suffix chains in
bf16 on the DVE (2x packed mode).  W3 is folded into the prefix seed,
coef_r into the per-r combine op.

Sharding: 8 cores = (batch b, query-half h); keys are permuted per core
so the core's own query half comes first (sum over keys is permutation
invariant), which halves the input DMA and transpose count.  The output
is produced transposed ([D, QH]) so the out-matmul can use natural-
layout X as the stationary operand; the host transposes back for free.

Engines: PE = transposes + q/k matmuls + R score matmuls/key-block +
out matmuls; ACT = tanh(k + s_r), sigmoid, kT/XT1 copies; DVE = prefix/
suffix chains + G finalize + XT0 copies; GPSIMD = X fp16 casts.
"""

import numpy as np

B, L, D, U = 4, 1024, 256, 128
QH = L // 2          # queries per core
N_CORES = 8

R = 16               # interpolation rank
CLIP = 5.0
NODES = [-5.0, -4.890738, -4.56772729, -4.04508497, -3.34565303, -2.5,
         -1.54508497, -0.522642316, 0.522642316, 1.54508497, 2.5,
         3.34565303, 4.04508497, 4.56772729, 4.890738, 5.0]
COEF = [-1.78875197e-08, 3.58280621e-08, -3.57950415e-08, 3.57870153e-08,
        -3.57927463e-08, 3.57886416e-08, -3.58003089e-08, 3.57928814e-08,
        -3.57928814e-08, 3.58003089e-08, -3.57886416e-08, 3.57927463e-08,
        -3.57870153e-08, 3.57950415e-08, -3.58280621e-08, 1.78875197e-08]

_CACHE = {}
LAST_RESULTS = None


def _build_program():
    import concourse.bass as bass
    import concourse.bacc as bacc
    import concourse.mybir as mybir
    import concourse.tile as tile
    from concourse import masks
    from concourse.alu_op_type import AluOpType as Alu

    f32 = mybir.dt.float32
    f16 = mybir.dt.float16
    bf16 = mybir.dt.bfloat16
    AF = mybir.ActivationFunctionType

    nc = bacc.Bacc(
        "TRN2",
        target_bir_lowering=False,
        debug=False,
        enable_asserts=False,
        num_devices=N_CORES,
    )

    Xb = nc.dram_tensor("Xb", [L, D], f32, kind="ExternalInput")
    W1 = nc.dram_tensor("W1", [D, U], f32, kind="ExternalInput")
    W2 = nc.dram_tensor("W2", [D, U], f32, kind="ExternalInput")
    W3 = nc.dram_tensor("W3v", [U, 1], f32, kind="ExternalInput")
    b1 = nc.dram_tensor("b1", [U, 1], f32, kind="ExternalInput")
    b2 = nc.dram_tensor("b2", [1, 1], f32, kind="ExternalInput")
    out = nc.dram_tensor("out", [D, QH], f32, kind="ExternalOutput")

    NLB = L // 128   # 8 key blocks
    NQB = QH // 128  # 4 query blocks per core
    NDB = D // 128   # 2 d blocks

    with tile.TileContext(nc) as tc:
        with (
            tc.tile_pool(name="const", bufs=1) as cp,
            tc.tile_pool(name="score_sb", bufs=2) as scp,
            tc.tile_pool(name="outs", bufs=2) as outp,
            tc.tile_pool(name="pre_ps", bufs=2, space="PSUM") as prepsum,
            tc.tile_pool(name="big_ps", bufs=3, space="PSUM") as bigpsum,
            tc.tile_pool(name="out_ps", bufs=1, space="PSUM") as outpsum,
        ):
            ident = cp.tile([128, 128], f32)
            masks.make_identity(nc, ident[:])

            # ---- input DMA; key blocks 0-3 are this core's queries ----
            Xs = cp.tile([128, NLB, D], f32)
            nc.sync.dma_start(
                Xs[:, 0:2, :], Xb[0:256].rearrange("(kb p) d -> p kb d", p=128))
            nc.scalar.dma_start(
                Xs[:, 2:4, :],
                Xb[256:512].rearrange("(kb p) d -> p kb d", p=128))
            W1s = cp.tile([128, NDB, U], f32)
            nc.sync.dma_start(W1s[:], W1[:].rearrange("(db p) u -> p db u", p=128))
            W2s = cp.tile([128, NDB, U], f32)
            nc.scalar.dma_start(W2s[:], W2[:].rearrange("(db p) u -> p db u", p=128))
            b1s = cp.tile([128, 1], f32)
            nc.scalar.dma_start(b1s[:], b1[:])
            W3s = cp.tile([128, 1], f32)
            nc.scalar.dma_start(W3s[:], W3[:])
            b2s = cp.tile([1, 1], f32)
            nc.scalar.dma_start(b2s[:], b2[:])
            nc.sync.dma_start(
                Xs[:, 4:6, :],
                Xb[512:768].rearrange("(kb p) d -> p kb d", p=128))
            nc.scalar.dma_start(
                Xs[:, 6:8, :],
                Xb[768:1024].rearrange("(kb p) d -> p kb d", p=128))

            ones1 = cp.tile([1, 128], f32)
            nc.vector.memset(ones1[:], 1.0)
            # per-node bias columns for the ACT tanh shifts
            nodecol = cp.tile([128, R], f32)
            for r in range(R):
                nc.gpsimd.memset(nodecol[:, r:r + 1], float(NODES[r]))

            # ---- transpose query half (key blocks 0-3) ----
            # copies split DVE/ACT to keep the q critical path short
            XT0 = cp.tile([128, NDB, QH], f32)
            for lb in range(4):
                for db in range(NDB):
                    tp = prepsum.tile([128, 128], f32, tag="pre")
                    nc.tensor.transpose(
                        tp[:], Xs[:, lb, db * 128:(db + 1) * 128], ident[:])
                    dst = XT0[:, db, lb * 128:(lb + 1) * 128]
                    if (lb * NDB + db) % 2 == 0:
                        nc.vector.tensor_copy(dst, tp[:])
                    else:
                        nc.scalar.activation(dst, tp[:], AF.Identity)

            # ---- q = W1^T XqT + b1, clipped, bf16 ----
            qpre = bigpsum.tile([128, QH], f32, tag="big")
            for db in range(NDB):
                nc.tensor.matmul(
                    qpre[:], W1s[:, db, :], XT0[:, db, :],
                    start=(db == 0), stop=(db == NDB - 1))
            qc = cp.tile([128, QH], bf16)
            nc.vector.tensor_scalar(
                qc[:], qpre[:], b1s[:], float(CLIP), Alu.add, Alu.min)
            nc.vector.tensor_scalar_max(qc[:], qc[:], float(-CLIP))

            # ---- kT half0 (keys 0-511): PE matmul, ACT copy to SBUF ----
            kpre0 = bigpsum.tile([128, QH], f32, tag="big")
            for db in range(NDB):
                nc.tensor.matmul(
                    kpre0[:], W2s[:, db, :], XT0[:, db, :],
                    start=(db == 0), stop=(db == NDB - 1))
            kT0 = cp.tile([128, QH], f32)
            nc.scalar.activation(kT0[:], kpre0[:], AF.Identity)

            # ---- b2 broadcast to all partitions ----
            tpb = prepsum.tile([128, 1], f32, tag="pre")
            nc.tensor.matmul(tpb[:], ones1[:], b2s[:])
            b2col = cp.tile([128, 1], f32)
            nc.scalar.activation(b2col[:], tpb[:], AF.Identity)

            # ---- transpose other half (key blocks 4-7), kT half1 ----
            XT1 = cp.tile([128, NDB, QH], f32)
            for lb in range(4):
                for db in range(NDB):
                    tp = prepsum.tile([128, 128], f32, tag="pre")
                    nc.tensor.transpose(
                        tp[:], Xs[:, 4 + lb, db * 128:(db + 1) * 128], ident[:])
                    nc.scalar.activation(
                        XT1[:, db, lb * 128:(lb + 1) * 128], tp[:], AF.Identity)
            kpre1 = bigpsum.tile([128, QH], f32, tag="big")
            for db in range(NDB):
                nc.tensor.matmul(
                    kpre1[:], W2s[:, db, :], XT1[:, db, :],
                    start=(db == 0), stop=(db == NDB - 1))

            # ---- prefix/suffix chains (bf16, DVE 2x mode) ----
            # pre[j] = W3 * prod_{i<=j} (qc - s_i);  suf[j] = prod_{i>=j}
            pre = cp.tile([128, R - 1, QH], bf16)
            suf = cp.tile([128, R - 1, QH], bf16)
            nc.vector.tensor_scalar(
                pre[:, 0, :], qc[:], float(-NODES[0]), W3s[:], Alu.add, Alu.mult)
            nc.vector.tensor_scalar_add(
                suf[:, R - 2, :], qc[:], float(-NODES[R - 1]))
            # interleave the two chains so neither blocks the other
            for step in range(1, R - 1):
                nc.vector.scalar_tensor_tensor(
                    pre[:, step, :], qc[:], float(-NODES[step]),
                    pre[:, step - 1, :], Alu.add, Alu.mult)
                nc.vector.scalar_tensor_tensor(
                    suf[:, R - 2 - step, :], qc[:],
                    float(-NODES[R - 1 - step]),
                    suf[:, R - 1 - step, :], Alu.add, Alu.mult)

            # ---- G_r = coef_r * W3 * prod_{j!=r}(qc - s_j)  (bf16) ----
            G = cp.tile([128, R, QH], bf16)
            for r in range(R):
                if r == 0:
                    nc.vector.tensor_scalar(
                        G[:, 0, :], suf[:, 0, :], W3s[:], float(COEF[0]),
                        Alu.mult, Alu.mult)
                elif r == R - 1:
                    nc.vector.tensor_scalar_mul(
                        G[:, r, :], pre[:, R - 2, :], float(COEF[r]))
                else:
                    nc.vector.scalar_tensor_tensor(
                        G[:, r, :], pre[:, r - 1, :], float(COEF[r]),
                        suf[:, r, :], Alu.mult, Alu.mult)

            # ---- H_r = tanh(kT + s_r) (bf16); first half on ACT now ----
            H = cp.tile([128, R, L], bf16)
            for r in range(R):
                nc.scalar.activation(
                    H[:, r, 0:QH], kT0[:], AF.Tanh, bias=nodecol[:, r:r + 1])
            kT1 = cp.tile([128, QH], f32)
            nc.scalar.activation(kT1[:], kpre1[:], AF.Identity)

            # ---- X in fp16 for the out matmuls (gpsimd, SBUF->SBUF) ----
            X16 = cp.tile([128, NLB, D], f16)
            nc.gpsimd.tensor_copy(X16[:, 0:4, :], Xs[:, 0:4, :])
            nc.gpsimd.tensor_copy(X16[:, 4:8, :], Xs[:, 4:8, :])

            # ---- main loop over key blocks ----
            # outT[d, q] accumulated in PSUM: stationary = X16 (natural
            # layout), moving = scT.  Output is [D, QH]; host transposes.
            poT = [outpsum.tile([128, QH], f32, tag=f"poT{db}", name=f"poT{db}")
                   for db in range(NDB)]

            def emit_score(kb, psT):
                scT = scp.tile([128, QH], f16, tag="scT", name="scT")
                nc.scalar.activation(scT[:], psT[:], AF.Sigmoid, bias=b2col[:])
                for db in range(NDB):
                    nc.tensor.matmul(
                        poT[db][:], X16[:, kb, db * 128:(db + 1) * 128],
                        scT[:],
                        start=(kb == 0), stop=(kb == NLB - 1),
                        skip_group_check=True)

            pending = None
            for kb in range(NLB):
                psT = bigpsum.tile([128, QH], f32, tag="big")
                for r in range(R):
                    nc.tensor.matmul(
                        psT[:], H[:, r, kb * 128:(kb + 1) * 128], G[:, r, :],
                        start=(r == 0), stop=(r == R - 1),
                        skip_group_check=True)
                if pending is not None:
                    emit_score(*pending)
                # second-half tanhs ride along during the first 4 blocks
                if kb < 4:
                    for r in range(4 * kb, 4 * kb + 4):
                        nc.scalar.activation(
                            H[:, r, QH:L], kT1[:], AF.Tanh,
                            bias=nodecol[:, r:r + 1])
                pending = None if kb == NLB - 1 else (kb, psT)
            emit_score(NLB - 1, psT)

            # ---- write out ----
            for db in range(NDB):
                ot = outp.tile([128, QH], f32, tag="ot", name="ot")
                nc.vector.tensor_copy(ot[:], poT[db][:])
                eng = nc.sync if db % 2 == 0 else nc.scalar
                eng.dma_start(out[db * 128:(db + 1) * 128, :], ot[:])

    nc.compile()
    return nc


def _get_nc():
    if "nc" not in _CACHE:
        _CACHE["nc"] = _build_program()
    return _CACHE["nc"]


def kernel(X, W1, W2, W3, bias1, bias2, trace=False):
    global LAST_RESULTS
    from concourse.bass_utils import run_bass_kernel_spmd

    X = np.ascontiguousarray(np.asarray(X, dtype=np.float32))
    W1 = np.ascontiguousarray(np.asarray(W1, dtype=np.float32))
    W2 = np.ascontiguousarray(np.asarray(W2, dtype=np.float32))
    W3 = np.ascontiguousarray(np.asarray(W3, dtype=np.float32))
    b1 = np.ascontiguousarray(np.asarray(bias1, dtype=np.float32).reshape(U, 1))
    b2 = np.ascontiguousarray(np.asarray(bias2, dtype=np.float32).reshape(1, 1))

    nc = _get_nc()
    in_maps = []
    for c in range(N_CORES):
        b, h = c // 2, c % 2
        if h == 0:
            Xbc = X[b]
        else:
            Xbc = np.ascontiguousarray(
                np.concatenate([X[b, QH:], X[b, :QH]], axis=0))
        in_maps.append({
            "Xb": Xbc,
            "W1": W1,
            "W2": W2,
            "W3v": W3,
            "b1": b1,
            "b2": b2,
        })

    res = run_bass_kernel_spmd(nc, in_maps, core_ids=list(range(N_CORES)),
                               trace=trace)
    LAST_RESULTS = res

    out = np.empty((B, L, D), dtype=np.float32)
    for c in range(N_CORES):
        b, h = c // 2, c % 2
        out[b, h * QH:(h + 1) * QH] = res.results[c]["out"].T
    return out


# revision 7
# speedup vs baseline: 1.2930x; 1.2930x over previous
"""Additive attention kernel for Trainium2 (8 NeuronCores, SPMD).

Reference computation (B=4, L=1024, D=256, U=128):
    q = X @ W1 + b1                              [B,L,U]
    k = X @ W2                                   [B,L,U]
    g = tanh(q[:,:,None,:] + k[:,None,:,:])      [B,L,L,U]
    s = sigmoid(g @ W3 + b2)                     [B,L,L]
    out = s @ X                                  [B,L,D]

Key idea: replace the O(L*L*U) tanh evaluation with a rank-R functional
decomposition (barycentric Lagrange interpolation in the tanh shift):

    tanh(q + k) ~= sum_r  coef_r * prod_{j!=r}(clip(q) - s_j) * tanh(k + s_r)

with s_r Chebyshev nodes on [-C, C].  The score computation becomes R
rank-U matmuls per key block on the PE instead of 67M tanh evals on ACT:

    psT[kk,q] = sum_r sum_u (W3_u ell_r(q_u)) * tanh(k_u(kk) + s_r)

Cost folding: all |coef_r| are ~alpha^(R-1) with alternating signs, so
each factor is scaled by alpha (folded into the d_j ops), the sign is
folded into the tanh via ACT's input scale (tanh(-k - s_r) = -tanh(k +
s_r)), and only the two endpoint terms need an explicit 0.5007 scale.
Leave-one-out products then come from division-free prefix/suffix
chains of pure tensor_tensor multiplies in bf16 on the DVE.

Input staging: the host passes fp16 copies of X/W1/W2; all transposes
run on the DMA xbar engine (dma_start_transpose), so the PE does no
transposes and ACT/DVE do no PSUM-copy work for them.  Keys are
permuted per core so the core's own query half comes first (the key
sum is permutation invariant).  The output is produced transposed
([D, QH]) so the out-matmul uses natural-layout fp16 X as stationary;
the host transposes back for free.
"""

import numpy as np

B, L, D, U = 4, 1024, 256, 128
QH = L // 2          # queries per core
N_CORES = 8

R = 14               # interpolation rank
CLIP = 4.5
ALPHA = 0.34591684387
NODES = [-4.5, -4.36923818, -3.98455212, -3.36829837, -2.55629136,
         -1.59572199, -0.542415061, 0.542415061, 1.59572199, 2.55629136,
         3.36829837, 3.98455212, 4.36923818, 4.5]
SIGNS = [-1, 1, -1, 1, -1, 1, -1, 1, -1, 1, -1, 1, -1, 1]
ENDSC = 0.500705131  # |beta| of the two endpoint terms

_CACHE = {}
LAST_RESULTS = None


def _build_program():
    import concourse.bass as bass
    import concourse.bacc as bacc
    import concourse.mybir as mybir
    import concourse.tile as tile
    from concourse.alu_op_type import AluOpType as Alu

    f32 = mybir.dt.float32
    f16 = mybir.dt.float16
    bf16 = mybir.dt.bfloat16
    AF = mybir.ActivationFunctionType

    nc = bacc.Bacc(
        "TRN2",
        target_bir_lowering=False,
        debug=False,
        enable_asserts=False,
        num_devices=N_CORES,
    )

    Xh = nc.dram_tensor("Xh", [L, D], f16, kind="ExternalInput")
    W1h = nc.dram_tensor("W1h", [D, U], f16, kind="ExternalInput")
    W2h = nc.dram_tensor("W2h", [D, U], f16, kind="ExternalInput")
    W3 = nc.dram_tensor("W3v", [U, 1], f32, kind="ExternalInput")
    b1 = nc.dram_tensor("b1", [U, 1], f32, kind="ExternalInput")
    b2 = nc.dram_tensor("b2", [1, 1], f32, kind="ExternalInput")
    out = nc.dram_tensor("out", [D, QH], f32, kind="ExternalOutput")

    NLB = L // 128   # 8 key blocks
    NDB = D // 128   # 2 d blocks

    with tile.TileContext(nc) as tc:
        with (
            tc.tile_pool(name="const", bufs=1) as cp,
            tc.tile_pool(name="score_sb", bufs=2) as scp,
            tc.tile_pool(name="outs", bufs=2) as outp,
            tc.tile_pool(name="big_ps", bufs=4, space="PSUM") as bigpsum,
            tc.tile_pool(name="out_ps", bufs=1, space="PSUM") as outpsum,
        ):
            # ---- input DMA ----
            # XT[u-part? no: d-part, db, keys] via DMA xbar transposes,
            # query half (keys 0-511) first on both queues
            XT = cp.tile([128, NDB, L], f16)
            W1s = cp.tile([128, NDB, U], f16)
            W2s = cp.tile([128, NDB, U], f16)
            nc.sync.dma_start(W1s[:], W1h[:].rearrange("(db p) u -> p db u", p=128))
            nc.scalar.dma_start(W2s[:], W2h[:].rearrange("(db p) u -> p db u", p=128))
            nc.sync.dma_start_transpose(XT[:, 0, 0:QH], Xh[0:QH, 0:128])
            nc.scalar.dma_start_transpose(XT[:, 1, 0:QH], Xh[0:QH, 128:256])
            W3s = cp.tile([128, 1], f32)
            nc.sync.dma_start(W3s[:], W3[:])
            b1s = cp.tile([128, 1], f32)
            nc.sync.dma_start(b1s[:], b1[:])
            b2s = cp.tile([1, 1], f32)
            nc.sync.dma_start(b2s[:], b2[:])
            nc.sync.dma_start_transpose(XT[:, 0, QH:L], Xh[QH:L, 0:128])
            nc.scalar.dma_start_transpose(XT[:, 1, QH:L], Xh[QH:L, 128:256])
            # natural-layout fp16 X for the out-matmul stationaries
            X16 = cp.tile([128, NLB, D], f16)
            nc.sync.dma_start(
                X16[:, 0:4, :], Xh[0:QH].rearrange("(kb p) d -> p kb d", p=128))
            nc.scalar.dma_start(
                X16[:, 4:8, :], Xh[QH:L].rearrange("(kb p) d -> p kb d", p=128))

            ones1 = cp.tile([1, 128], f32)
            nc.vector.memset(ones1[:], 1.0)
            # bias columns (sign_r * s_r) and the -1 scale column
            nodecol = cp.tile([128, R], f32)
            for r in range(R):
                nc.gpsimd.memset(nodecol[:, r:r + 1], float(SIGNS[r] * NODES[r]))
            negcol = cp.tile([128, 1], f32)
            nc.gpsimd.memset(negcol[:], -1.0)

            # ---- q = W1^T XqT + b1, clipped, bf16 ----
            qpre = bigpsum.tile([128, QH], f32, tag="big")
            for db in range(NDB):
                nc.tensor.matmul(
                    qpre[:], W1s[:, db, :], XT[:, db, 0:QH],
                    start=(db == 0), stop=(db == NDB - 1))
            qc = cp.tile([128, QH], bf16)
            nc.vector.tensor_scalar(
                qc[:], qpre[:], b1s[:], float(CLIP), Alu.add, Alu.min)
            nc.vector.tensor_scalar_max(qc[:], qc[:], float(-CLIP))
            aW3 = cp.tile([128, 1], f32)
            nc.vector.tensor_scalar_mul(aW3[:], W3s[:], float(ALPHA))

            # ---- kT: two halves on PE, copied to SBUF by ACT ----
            kT = cp.tile([128, 2, QH], f32)
            kpre = []
            for lh in range(2):
                kp = bigpsum.tile([128, QH], f32, tag="big")
                for db in range(NDB):
                    nc.tensor.matmul(
                        kp[:], W2s[:, db, :], XT[:, db, lh * QH:(lh + 1) * QH],
                        start=(db == 0), stop=(db == NDB - 1))
                nc.scalar.activation(kT[:, lh, :], kp[:], AF.Identity)

            # ---- b2 broadcast to all partitions ----
            tpb = bigpsum.tile([128, 1], f32, tag="big")
            nc.tensor.matmul(tpb[:], ones1[:], b2s[:])
            b2col = cp.tile([128, 1], f32)
            nc.scalar.activation(b2col[:], tpb[:], AF.Identity)

            # ---- d_j = alpha*(qc - s_j), bf16, fast single-src ops ----
            dd = cp.tile([128, R, QH], bf16)
            for j in range(1, R - 1):
                nc.vector.tensor_scalar(
                    dd[:, j, :], qc[:], float(-NODES[j]), float(ALPHA),
                    Alu.add, Alu.mult)

            # ---- prefix/suffix chains (pure tensor_tensor, bf16) ----
            pre = cp.tile([128, R - 1, QH], bf16)
            suf = cp.tile([128, R - 1, QH], bf16)
            nc.vector.tensor_scalar(
                pre[:, 0, :], qc[:], float(-NODES[0]), aW3[:], Alu.add, Alu.mult)
            nc.vector.tensor_scalar(
                suf[:, R - 2, :], qc[:], float(-NODES[R - 1]), float(ALPHA),
                Alu.add, Alu.mult)
            for step in range(1, R - 1):
                nc.vector.tensor_tensor(
                    pre[:, step, :], pre[:, step - 1, :], dd[:, step, :],
                    Alu.mult)
                nc.vector.tensor_tensor(
                    suf[:, R - 2 - step, :], suf[:, R - 1 - step, :],
                    dd[:, R - 1 - step, :], Alu.mult)

            # ---- G_r (bf16): pure products; ends get the 0.5007 scale ----
            G = cp.tile([128, R, QH], bf16)
            for r in range(R):
                if r == 0:
                    nc.vector.tensor_scalar(
                        G[:, 0, :], suf[:, 0, :], W3s[:], float(ENDSC),
                        Alu.mult, Alu.mult)
                elif r == R - 1:
                    nc.vector.tensor_scalar_mul(
                        G[:, r, :], pre[:, R - 2, :], float(ENDSC))
                else:
                    nc.vector.tensor_tensor(
                        G[:, r, :], pre[:, r - 1, :], suf[:, r, :], Alu.mult)

            # ---- H_r = sign_r*tanh(kT + s_r) = tanh(sign_r*kT + sign_r*s_r)
            H = cp.tile([128, R, L], bf16)
            for r in range(R):
                kwargs = {"bias": nodecol[:, r:r + 1]}
                if SIGNS[r] < 0:
                    kwargs["scale"] = negcol[:]
                nc.scalar.activation(H[:, r, 0:QH], kT[:, 0, :], AF.Tanh,
                                     **kwargs)

            # ---- main loop over key blocks ----
            poT = [outpsum.tile([128, QH], f32, tag=f"poT{db}", name=f"poT{db}")
                   for db in range(NDB)]

            def emit_score(kb, psT):
                scT = scp.tile([128, QH], f16, tag="scT", name="scT")
                nc.scalar.activation(scT[:], psT[:], AF.Sigmoid, bias=b2col[:])
                for db in range(NDB):
                    nc.tensor.matmul(
                        poT[db][:], X16[:, kb, db * 128:(db + 1) * 128],
                        scT[:],
                        start=(kb == 0), stop=(kb == NLB - 1),
                        skip_group_check=True)

            # second-half tanh emission schedule: groups of 4,4,3,3
            hb_groups = [list(range(0, 4)), list(range(4, 8)),
                         list(range(8, 11)), list(range(11, 14))]
            pending = None
            for kb in range(NLB):
                psT = bigpsum.tile([128, QH], f32, tag="big")
                for r in range(R):
                    nc.tensor.matmul(
                        psT[:], H[:, r, kb * 128:(kb + 1) * 128], G[:, r, :],
                        start=(r == 0), stop=(r == R - 1),
                        skip_group_check=True)
                if pending is not None:
                    emit_score(*pending)
                if kb < 4:
                    for r in hb_groups[kb]:
                        kwargs = {"bias": nodecol[:, r:r + 1]}
                        if SIGNS[r] < 0:
                            kwargs["scale"] = negcol[:]
                        nc.scalar.activation(H[:, r, QH:L], kT[:, 1, :],
                                             AF.Tanh, **kwargs)
                pending = None if kb == NLB - 1 else (kb, psT)
            emit_score(NLB - 1, psT)

            # ---- write out ----
            for db in range(NDB):
                ot = outp.tile([128, QH], f32, tag="ot", name="ot")
                nc.vector.tensor_copy(ot[:], poT[db][:])
                eng = nc.sync if db % 2 == 0 else nc.scalar
                eng.dma_start(out[db * 128:(db + 1) * 128, :], ot[:])

    nc.compile()
    return nc


def _get_nc():
    if "nc" not in _CACHE:
        _CACHE["nc"] = _build_program()
    return _CACHE["nc"]


def kernel(X, W1, W2, W3, bias1, bias2, trace=False):
    global LAST_RESULTS
    from concourse.bass_utils import run_bass_kernel_spmd

    X = np.ascontiguousarray(np.asarray(X, dtype=np.float32))
    W1h = np.ascontiguousarray(np.asarray(W1, dtype=np.float16))
    W2h = np.ascontiguousarray(np.asarray(W2, dtype=np.float16))
    W3 = np.ascontiguousarray(np.asarray(W3, dtype=np.float32))
    b1 = np.ascontiguousarray(np.asarray(bias1, dtype=np.float32).reshape(U, 1))
    b2 = np.ascontiguousarray(np.asarray(bias2, dtype=np.float32).reshape(1, 1))

    nc = _get_nc()
    in_maps = []
    for c in range(N_CORES):
        b, h = c // 2, c % 2
        if h == 0:
            Xbc = X[b]
        else:
            Xbc = np.concatenate([X[b, QH:], X[b, :QH]], axis=0)
        in_maps.append({
            "Xh": np.ascontiguousarray(Xbc.astype(np.float16)),
            "W1h": W1h,
            "W2h": W2h,
            "W3v": W3,
            "b1": b1,
            "b2": b2,
        })

    res = run_bass_kernel_spmd(nc, in_maps, core_ids=list(range(N_CORES)),
                               trace=trace)
    LAST_RESULTS = res

    out = np.empty((B, L, D), dtype=np.float32)
    for c in range(N_CORES):
        b, h = c // 2, c % 2
        out[b, h * QH:(h + 1) * QH] = res.results[c]["out"].T
    return out


# revision 8
# speedup vs baseline: 1.5124x; 1.1697x over previous
"""Additive attention kernel for Trainium2 (8 NeuronCores, SPMD).

Reference computation (B=4, L=1024, D=256, U=128):
    q = X @ W1 + b1                              [B,L,U]
    k = X @ W2                                   [B,L,U]
    g = tanh(q[:,:,None,:] + k[:,None,:,:])      [B,L,L,U]
    s = sigmoid(g @ W3 + b2)                     [B,L,L]
    out = s @ X                                  [B,L,D]

Key idea: replace the O(L*L*U) tanh evaluation with a rank-R functional
decomposition (barycentric Lagrange interpolation in the tanh shift):

    tanh(q + k) ~= sum_r  coef_r * prod_{j!=r}(clip(q) - s_j) * tanh(k + s_r)

with s_r Chebyshev nodes on [-C, C].  The score computation becomes R
rank-U matmuls per key block on the PE instead of 67M tanh evals on ACT:

    psT[kk,q] = sum_r sum_u (W3_u ell_r(q_u)) * tanh(k_u(kk) + s_r)

Cost folding: all |coef_r| are ~alpha^(R-1) with alternating signs, so
each factor is scaled by alpha (folded into the d_j ops), the sign is
folded into the tanh via ACT's input scale (tanh(-k - s_r) = -tanh(k +
s_r)), and only the two endpoint terms need an explicit 0.5007 scale.
Leave-one-out products then come from division-free prefix/suffix
chains of pure tensor_tensor multiplies in bf16 on the DVE.

Input staging: the host passes fp16 copies of X/W1/W2; all transposes
run on the DMA xbar engine (dma_start_transpose), so the PE does no
transposes and ACT/DVE do no PSUM-copy work for them.  Keys are
permuted per core so the core's own query half comes first (the key
sum is permutation invariant).  The output is produced transposed
([D, QH]) so the out-matmul uses natural-layout fp16 X as stationary;
the host transposes back for free.
"""

import numpy as np

B, L, D, U = 4, 1024, 256, 128
QH = L // 2          # queries per core
N_CORES = 8

R = 14               # interpolation rank
CLIP = 4.5
ALPHA = 0.34591684387
NODES = [-4.5, -4.36923818, -3.98455212, -3.36829837, -2.55629136,
         -1.59572199, -0.542415061, 0.542415061, 1.59572199, 2.55629136,
         3.36829837, 3.98455212, 4.36923818, 4.5]
SIGNS = [-1, 1, -1, 1, -1, 1, -1, 1, -1, 1, -1, 1, -1, 1]
ENDSC = 0.500705131  # |beta| of the two endpoint terms

_CACHE = {}
LAST_RESULTS = None


def _build_program():
    import concourse.bass as bass
    import concourse.bacc as bacc
    import concourse.mybir as mybir
    import concourse.tile as tile
    from concourse.alu_op_type import AluOpType as Alu

    f32 = mybir.dt.float32
    f16 = mybir.dt.float16
    bf16 = mybir.dt.bfloat16
    AF = mybir.ActivationFunctionType

    nc = bacc.Bacc(
        "TRN2",
        target_bir_lowering=False,
        debug=False,
        enable_asserts=False,
        num_devices=N_CORES,
    )

    Xh = nc.dram_tensor("Xh", [L, D], f16, kind="ExternalInput")
    XhT = nc.dram_tensor("XhT", [D, L], f16, kind="ExternalInput")
    W1h = nc.dram_tensor("W1h", [D, U], f16, kind="ExternalInput")
    W2h = nc.dram_tensor("W2h", [D, U], f16, kind="ExternalInput")
    W3 = nc.dram_tensor("W3v", [U, 1], f32, kind="ExternalInput")
    b1 = nc.dram_tensor("b1", [U, 1], f32, kind="ExternalInput")
    b2 = nc.dram_tensor("b2", [1, 1], f32, kind="ExternalInput")
    out = nc.dram_tensor("out", [D, QH], f32, kind="ExternalOutput")

    NLB = L // 128   # 8 key blocks
    NDB = D // 128   # 2 d blocks

    with tile.TileContext(nc) as tc:
        with (
            tc.tile_pool(name="const", bufs=1) as cp,
            tc.tile_pool(name="score_sb", bufs=2) as scp,
            tc.tile_pool(name="outs", bufs=2) as outp,
            tc.tile_pool(name="big_ps", bufs=4, space="PSUM") as bigpsum,
            tc.tile_pool(name="out_ps", bufs=1, space="PSUM") as outpsum,
        ):
            # ---- input DMA ----
            # XT[u-part? no: d-part, db, keys] via DMA xbar transposes,
            # query half (keys 0-511) first on both queues
            XT = cp.tile([128, NDB, L], f16)
            W1s = cp.tile([128, NDB, U], f16)
            W2s = cp.tile([128, NDB, U], f16)
            W3s = cp.tile([128, 1], f32)
            b1s = cp.tile([128, 1], f32)
            b2s = cp.tile([1, 1], f32)
            nc.sync.dma_start(W3s[:], W3[:])
            nc.sync.dma_start(b1s[:], b1[:])
            nc.sync.dma_start(b2s[:], b2[:])
            nc.sync.dma_start(W1s[:], W1h[:].rearrange("(db p) u -> p db u", p=128))
            nc.scalar.dma_start(W2s[:], W2h[:].rearrange("(db p) u -> p db u", p=128))
            # query-half columns of X^T first (q critical path)
            nc.sync.dma_start(
                XT[:, :, 0:QH],
                XhT[:, 0:QH].rearrange("(db p) k -> p db k", p=128))
            nc.scalar.dma_start(
                XT[:, :, QH:L],
                XhT[:, QH:L].rearrange("(db p) k -> p db k", p=128))
            # natural-layout fp16 X for the out-matmul stationaries
            X16 = cp.tile([128, NLB, D], f16)
            nc.sync.dma_start(
                X16[:, 0:4, :], Xh[0:QH].rearrange("(kb p) d -> p kb d", p=128))
            nc.scalar.dma_start(
                X16[:, 4:8, :], Xh[QH:L].rearrange("(kb p) d -> p kb d", p=128))

            ones1 = cp.tile([1, 128], f32)
            nc.vector.memset(ones1[:], 1.0)
            # bias columns (sign_r * s_r) and the -1 scale column
            nodecol = cp.tile([128, R], f32)
            for r in range(R):
                nc.gpsimd.memset(nodecol[:, r:r + 1], float(SIGNS[r] * NODES[r]))
            negcol = cp.tile([128, 1], f32)
            nc.gpsimd.memset(negcol[:], -1.0)

            # ---- q = W1^T XqT + b1, clipped, bf16 ----
            qpre = bigpsum.tile([128, QH], f32, tag="big")
            for db in range(NDB):
                nc.tensor.matmul(
                    qpre[:], W1s[:, db, :], XT[:, db, 0:QH],
                    start=(db == 0), stop=(db == NDB - 1))
            qc = cp.tile([128, QH], bf16)
            nc.vector.tensor_scalar(
                qc[:], qpre[:], b1s[:], float(CLIP), Alu.add, Alu.min)
            nc.vector.tensor_scalar_max(qc[:], qc[:], float(-CLIP))
            aW3 = cp.tile([128, 1], f32)
            nc.vector.tensor_scalar_mul(aW3[:], W3s[:], float(ALPHA))

            # ---- kT: two halves on PE, copied to SBUF by ACT ----
            kT = cp.tile([128, 2, QH], f32)
            kpre = []
            for lh in range(2):
                kp = bigpsum.tile([128, QH], f32, tag="big")
                for db in range(NDB):
                    nc.tensor.matmul(
                        kp[:], W2s[:, db, :], XT[:, db, lh * QH:(lh + 1) * QH],
                        start=(db == 0), stop=(db == NDB - 1))
                nc.scalar.activation(kT[:, lh, :], kp[:], AF.Identity)

            # ---- b2 broadcast to all partitions ----
            tpb = bigpsum.tile([128, 1], f32, tag="big")
            nc.tensor.matmul(tpb[:], ones1[:], b2s[:])
            b2col = cp.tile([128, 1], f32)
            nc.scalar.activation(b2col[:], tpb[:], AF.Identity)

            # ---- prefix/suffix chains with interleaved d_j and G_r ----
            # d_j = alpha*(qc - s_j); after chain step s the tiles pre_s
            # and suf_{R-2-s} exist, so G_r (= pre_{r-1}*suf_r) is emitted
            # middle-out as soon as both inputs exist.  The PE consumes
            # the G_r in the same order (R_ORDER).
            dd = cp.tile([128, R, QH], bf16)
            pre = cp.tile([128, R - 1, QH], bf16)
            suf = cp.tile([128, R - 1, QH], bf16)
            G = cp.tile([128, R, QH], bf16)

            def emit_G(r):
                if r == 0:
                    nc.vector.tensor_scalar(
                        G[:, 0, :], suf[:, 0, :], W3s[:], float(ENDSC),
                        Alu.mult, Alu.mult)
                elif r == R - 1:
                    nc.vector.tensor_scalar_mul(
                        G[:, r, :], pre[:, R - 2, :], float(ENDSC))
                else:
                    nc.vector.tensor_tensor(
                        G[:, r, :], pre[:, r - 1, :], suf[:, r, :], Alu.mult)

            nc.vector.tensor_scalar(
                pre[:, 0, :], qc[:], float(-NODES[0]), aW3[:], Alu.add, Alu.mult)
            nc.vector.tensor_scalar(
                suf[:, R - 2, :], qc[:], float(-NODES[R - 1]), float(ALPHA),
                Alu.add, Alu.mult)
            ready = set()
            ready_G = set()
            R_ORDER = []
            for step in range(1, R - 1):
                for j in (step, R - 1 - step):
                    if j not in ready:
                        nc.vector.tensor_scalar(
                            dd[:, j, :], qc[:], float(-NODES[j]), float(ALPHA),
                            Alu.add, Alu.mult)
                        ready.add(j)
                nc.vector.tensor_tensor(
                    pre[:, step, :], pre[:, step - 1, :], dd[:, step, :],
                    Alu.mult)
                nc.vector.tensor_tensor(
                    suf[:, R - 2 - step, :], suf[:, R - 1 - step, :],
                    dd[:, R - 1 - step, :], Alu.mult)
                for r in range(1, R - 1):
                    if r not in ready_G and max(r - 1, R - 2 - r) <= step:
                        emit_G(r)
                        ready_G.add(r)
                        R_ORDER.append(r)
            emit_G(R - 1)
            R_ORDER.append(R - 1)
            emit_G(0)
            R_ORDER.append(0)

            # ---- H_r = sign_r*tanh(kT + s_r) = tanh(sign_r*kT + sign_r*s_r)
            H = cp.tile([128, R, L], bf16)
            for lh in range(2):
                for r in range(R):
                    kwargs = {"bias": nodecol[:, r:r + 1]}
                    if SIGNS[r] < 0:
                        kwargs["scale"] = negcol[:]
                    nc.scalar.activation(
                        H[:, r, lh * QH:(lh + 1) * QH], kT[:, lh, :], AF.Tanh,
                        **kwargs)

            # ---- main loop over key blocks ----
            poT = [outpsum.tile([128, QH], f32, tag=f"poT{db}", name=f"poT{db}")
                   for db in range(NDB)]

            def emit_score(kb, psT):
                scT = scp.tile([128, QH], f16, tag="scT", name="scT")
                nc.scalar.activation(scT[:], psT[:], AF.Sigmoid, bias=b2col[:])
                for db in range(NDB):
                    nc.tensor.matmul(
                        poT[db][:], X16[:, kb, db * 128:(db + 1) * 128],
                        scT[:],
                        start=(kb == 0), stop=(kb == NLB - 1),
                        skip_group_check=True)

            pending = None
            for kb in range(NLB):
                psT = bigpsum.tile([128, QH], f32, tag="big")
                for i, r in enumerate(R_ORDER):
                    nc.tensor.matmul(
                        psT[:], H[:, r, kb * 128:(kb + 1) * 128], G[:, r, :],
                        start=(i == 0), stop=(i == R - 1),
                        skip_group_check=True)
                if pending is not None:
                    emit_score(*pending)
                pending = None if kb == NLB - 1 else (kb, psT)
            emit_score(NLB - 1, psT)

            # ---- write out ----
            for db in range(NDB):
                ot = outp.tile([128, QH], f32, tag="ot", name="ot")
                nc.vector.tensor_copy(ot[:], poT[db][:])
                eng = nc.sync if db % 2 == 0 else nc.scalar
                eng.dma_start(out[db * 128:(db + 1) * 128, :], ot[:])

    nc.compile()
    return nc


def _get_nc():
    if "nc" not in _CACHE:
        _CACHE["nc"] = _build_program()
    return _CACHE["nc"]


def kernel(X, W1, W2, W3, bias1, bias2, trace=False):
    global LAST_RESULTS
    from concourse.bass_utils import run_bass_kernel_spmd

    X = np.ascontiguousarray(np.asarray(X, dtype=np.float32))
    W1h = np.ascontiguousarray(np.asarray(W1, dtype=np.float16))
    W2h = np.ascontiguousarray(np.asarray(W2, dtype=np.float16))
    W3 = np.ascontiguousarray(np.asarray(W3, dtype=np.float32))
    b1 = np.ascontiguousarray(np.asarray(bias1, dtype=np.float32).reshape(U, 1))
    b2 = np.ascontiguousarray(np.asarray(bias2, dtype=np.float32).reshape(1, 1))

    nc = _get_nc()
    in_maps = []
    for c in range(N_CORES):
        b, h = c // 2, c % 2
        if h == 0:
            Xbc = X[b]
        else:
            Xbc = np.concatenate([X[b, QH:], X[b, :QH]], axis=0)
        Xbc16 = Xbc.astype(np.float16)
        in_maps.append({
            "Xh": np.ascontiguousarray(Xbc16),
            "XhT": np.ascontiguousarray(Xbc16.T),
            "W1h": W1h,
            "W2h": W2h,
            "W3v": W3,
            "b1": b1,
            "b2": b2,
        })

    res = run_bass_kernel_spmd(nc, in_maps, core_ids=list(range(N_CORES)),
                               trace=trace)
    LAST_RESULTS = res

    out = np.empty((B, L, D), dtype=np.float32)
    for c in range(N_CORES):
        b, h = c // 2, c % 2
        out[b, h * QH:(h + 1) * QH] = res.results[c]["out"].T
    return out
